# revision 1
# baseline (speedup 1.0000x reference)
"""Embedding lookup (nn.Embedding) on 8 Trainium2 NeuronCores.

Strategy: data-parallel shard token_ids along the batch dim (8 batch rows ->
8 cores), replicate the [50257, 1024] f32 table to every core's DRAM.
Each core gathers its 4096 rows with SWDGE indirect DMA (DRAM table -> SBUF)
and streams the gathered data back out to DRAM with HWDGE writes.

Hardware constraints found by probing (CoreSim is more permissive than the
real walrus/NRT stack):
  - walrus allows at most ONE sync wait attached to a DMA instruction and
    only a few on Tile's auto-generated tail Drain -> use the raw Block API
    with explicit semaphores; waits become standalone sequencer instructions.
  - the indirect-DMA offset AP must be [P, 1] (one index per partition);
    multi-column offset APs hang the device.
  - the indirect-DMA destination must be a whole SBUF tensor at offset 0;
    sliced destinations gather into the wrong place. The 32 per-column dest
    tiles are therefore aliases (alloc_sbuf_tensor_at) into one contiguous
    arena, so writes can still read multi-column spans with large
    contiguous descriptors.
  - shared-semaphore waits are only unambiguous at full multiples of
    16 * n_ops (SDMA engines complete in-flight ops out of order).

Per-core HBM traffic: 16 MB gather read + 16 MB output write  ->  ~90 us
roofline at ~360 GB/s shared read+write bandwidth.
"""

import numpy as np

from concourse import bass, mybir
from concourse.bass_utils import run_bass_kernel_spmd

VOCAB = 50257
D = 1024
B = 8
S = 4096
N_CORES = 8
P = 128
COLS = S // P  # 32 token columns per core (one token per partition per column)

# Columns per write group: each write spans W gathered columns -> W*4KB
# contiguous descriptors per partition. W=1 reproduces the per-column
# baseline; larger W trades write-start latency for descriptor efficiency.
W_GROUP = 2


def build_module(vocab=VOCAB, d=D, cols=COLS, w_group=W_GROUP):
    """One SPMD Bass program: [P, cols] int32 token ids -> [P, cols, d] f32."""
    assert cols % w_group == 0
    n_grp = cols // w_group
    # detect_race_conditions=False: CoreSim's conservative checker flags the
    # intentional arena aliasing (semaphores order every access correctly)
    nc = bass.Bass("TRN2", enable_partition_id=False, detect_race_conditions=False)
    tok = nc.declare_dram_parameter("token_ids", [P, cols], mybir.dt.int32, isOutput=False)
    w = nc.declare_dram_parameter("weight", [vocab, d], mybir.dt.float32, isOutput=False)
    out = nc.declare_dram_parameter("out", [P, cols, d], mybir.dt.float32, isOutput=True)

    row_bytes = d * 4

    with (
        nc.Block() as block,
        nc.semaphore("idx_sem") as idx_sem,
        nc.semaphore("w_sem") as w_sem,
    ):
        # manual allocations, never freed (stack-order free assert)
        idx = nc.alloc_sbuf_tensor("idx", [P, cols], mybir.dt.int32)
        gbig = nc.alloc_sbuf_tensor("gbig", [P, cols * d], mybir.dt.float32)
        base = nc.lookup_mloc(gbig).addr
        # per-column whole-tensor aliases into the arena (indirect-DMA dests)
        tiles = [
            nc.alloc_sbuf_tensor_at(
                f"ga{c}", [P, d], mybir.dt.float32, offset=base + c * row_bytes
            )
            for c in range(cols)
        ]
        g_sems = [nc.semaphore(f"g_sem{i}").__enter__() for i in range(n_grp)]

        @block.gpsimd
        def _(g: bass.BassEngine):
            g.wait_ge(idx_sem, 16)
            for c in range(cols):
                # index at (p, c) selects the table row landing in tile c row p
                g.indirect_dma_start(
                    out=tiles[c][:],
                    out_offset=None,
                    in_=w[:],
                    in_offset=bass.IndirectOffsetOnAxis(ap=idx[:, c : c + 1], axis=0),
                ).then_inc(g_sems[c // w_group], 16)

        @block.sync
        def _(s: bass.BassEngine):
            s.dma_start(out=idx[:], in_=tok[:]).then_inc(idx_sem, 16)
            for gi in range(n_grp):
                lo = gi * w_group
                hi = lo + w_group
                s.wait_ge(g_sems[gi], 16 * w_group)
                s.dma_start(
                    out=out[:, lo:hi, :], in_=gbig[:, lo * d : hi * d]
                ).then_inc(w_sem, 16)
            # total completion: every SDMA engine finished every write
            s.wait_ge(w_sem, 16 * n_grp)

    return nc


_module_cache = {}


def _get_module():
    if "m" not in _module_cache:
        _module_cache["m"] = build_module()
    return _module_cache["m"]


def kernel(token_ids, weight, **run_kwargs):
    token_ids = np.asarray(token_ids)
    weight = np.asarray(weight, dtype=np.float32)
    assert token_ids.shape == (B, S), token_ids.shape
    assert weight.shape == (VOCAB, D), weight.shape
    ids32 = np.ascontiguousarray(token_ids.astype(np.int32))

    nc = _get_module()
    # idx[p, c] = flat token p*COLS + c; out[p, c] likewise -> plain reshape
    in_maps = [
        {"token_ids": ids32[i].reshape(P, COLS), "weight": weight}
        for i in range(N_CORES)
    ]
    res = run_bass_kernel_spmd(nc, in_maps, core_ids=list(range(N_CORES)), **run_kwargs)
    out = np.stack(
        [res.results[i]["out"].reshape(S, D) for i in range(N_CORES)]
    ).reshape(B, S, D)
    if run_kwargs:
        return out, res
    return out

